# revision 42
# baseline (speedup 1.0000x reference)
"""
Trainium2 Bass kernel for nn_LocalAttention2d (sparse local attention with
predictive alignment).

Strategy (pure data parallel, B=8 batches over 8 NeuronCores):
  host:   per batch, build a zero-padded channels-last fp16 image
          qcl[(H+7)*(W+7), Cq]; transpose c_t -> [Cc, T].
  device: PE computes Wc = c_t @ W_a and the alignment MLP p_t
          (sigmoid via tanh identity to stay in one ACT table set).
          Window-row indices are computed directly in the SWDGE wrapped
          [16, ...] index layout via one small DRAM round trip.  One
          dma_gather descriptor fetches a whole 7-pixel window row
          (elem_step=256 < elem_size=1792, rows overlap in DRAM), so each
          128-query block needs just 896 descriptors of 3.5KB.
          Scores: fused scalar_tensor_tensor multiply + accumulate per
          window slot (fp16 data, fp32 accumulation).  Masked softmax via
          the (s+BIG)*valid trick.  Combine: per-slot scaled copies split
          between ACT (activation scale) and DVE (tensor_scalar), then a
          strided pairwise tree reduction on DVE, all fp16.
"""

import numpy as np

import concourse.bass as bass
import concourse.bacc as bacc
import concourse.mybir as mybir
import concourse.tile as tile
from concourse.bass_utils import run_bass_kernel_spmd

f32 = mybir.dt.float32
f16 = mybir.dt.float16
i32 = mybir.dt.int32
i16 = mybir.dt.int16
AF = mybir.ActivationFunctionType
ALU = mybir.AluOpType
AX = mybir.AxisListType

# Problem constants (hardcoded per contract)
B, Cq, H, W = 8, 256, 128, 128
T, Cc, P = 1024, 256, 128
R = 3
K7 = 2 * R + 1          # 7
K = K7 * K7             # 49
PADL = 3                # original pixel (y,x) -> padded (y+3, x+3)
HP = H + K7             # 135
WP = W + K7             # 135
NPOS = HP * WP          # 18225
TB = 128                # queries per block
NB = T // TB            # 8 blocks
ROWLEN = K7 * Cq        # 1792 elems per gathered window row
MAGIC = 8388608.0       # 2^23: fp32 add/sub rounds half-even like jnp.round
BIG = 1024.0            # mask offset; exp(-~1000) == 0
ACT_SLOTS = 44          # combine slots scaled on ACT; rest on DVE
CH1 = 30                # ACT slots emitted before the next block's exp

_CACHE = {}


def _build_nc():
    nc = bacc.Bacc(None, target_bir_lowering=False, num_swdge_queues=4)

    qcl = nc.dram_tensor("qcl", [NPOS, Cq], f16, kind="ExternalInput")
    ctT = nc.dram_tensor("ctT", [Cc, T], f32, kind="ExternalInput")
    wa = nc.dram_tensor("wa", [Cc, Cq], f32, kind="ExternalInput")
    wpT = nc.dram_tensor("wpT", [Cc, P], f32, kind="ExternalInput")
    vpb = nc.dram_tensor("vpb", [2 * P, P], f32, kind="ExternalInput")
    offs = nc.dram_tensor("offs", [128, K7], f32, kind="ExternalInput")
    io135 = nc.dram_tensor("io135", [16, K7], f32, kind="ExternalInput")
    out_d = nc.dram_tensor("out", [T, Cq], f32, kind="ExternalOutput")
    rnd_d = nc.dram_tensor("rnd_stage", [128, 2 * NB], f32, kind="Internal")
    idx_d = nc.dram_tensor("idx_stage", [128, NB * K7 * 8], i16, kind="Internal")

    def ap_with(apx, dims, doff=0):
        """AP reusing apx's partition dim, explicit free dims, +elem offset."""
        return bass.AP(apx.tensor, apx.offset + doff,
                       [apx.ap[0]] + [list(d) for d in dims])

    with tile.TileContext(nc) as tc:
        with (
            tc.tile_pool(name="const", bufs=1) as cp,
            tc.tile_pool(name="work", bufs=2) as wk,
            tc.tile_pool(name="patch", bufs=4) as pp,
            tc.tile_pool(name="prodp", bufs=1) as prp,
            tc.tile_pool(name="psum", bufs=4, space="PSUM") as ps,
        ):
            # ---- load weights/constants ----
            ct_sb = cp.tile([128, 2, T], f32)
            wa_sb = cp.tile([128, 2, Cq], f32)
            wpT_sb = cp.tile([128, 2, P], f32)
            vpb_sb = cp.tile([128, 2, P], f32)
            offs_sb = cp.tile([128, K7], f32)
            io_sb = cp.tile([16, K7], f32)
            for h in range(2):
                nc.sync.dma_start(ct_sb[:, h, :], ctT[h * 128:(h + 1) * 128, :])
                nc.sync.dma_start(wpT_sb[:, h, :], wpT[h * 128:(h + 1) * 128, :])
                nc.sync.dma_start(wa_sb[:, h, :], wa[h * 128:(h + 1) * 128, :])
                nc.sync.dma_start(vpb_sb[:, h, :], vpb[h * 128:(h + 1) * 128, :])
            nc.sync.dma_start(offs_sb[:], offs[:])
            nc.sync.dma_start(io_sb[:], io135[:])

            # persistent per-block state
            wc_sb = cp.tile([128, NB, Cq], f16)      # Wc per block (fp16)
            valid_sb = cp.tile([128, NB, K], f32)    # slot validity
            elog_sb = cp.tile([128, NB, K], f32)     # log gaussian decay
            h_all = cp.tile([128, NB, P], f32)       # tanh(c_t @ W_p.T)
            pz = cp.tile([128, NB, 2], f32)          # raw V_p dots
            th = cp.tile([128, NB, 2], f32)          # tanh(z/2)
            p_all = cp.tile([128, NB, 2], f32)       # p_t
            rm_all = cp.tile([128, NB, 2], f32)      # p + MAGIC
            rnd_all = cp.tile([128, NB, 2], f32)     # round(p)
            idxw = cp.tile([128, NB * K7 * 8], i16)  # wrapped gather indices

            # ---- alignment MLP: h = tanh(ct @ WpT), z = h . Vp ----
            for b in range(NB):
                blk = slice(b * 128, (b + 1) * 128)
                acch = ps.tile([128, P], f32, tag="mmh")
                nc.tensor.matmul(acch[:], ct_sb[:, 0, blk], wpT_sb[:, 0, :],
                                 start=True, stop=False)
                nc.tensor.matmul(acch[:], ct_sb[:, 1, blk], wpT_sb[:, 1, :],
                                 start=False, stop=True)
                nc.scalar.activation(h_all[:, b, :], acch[:], AF.Tanh)
                for u in range(2):
                    sj = wk.tile([128, P], f16, tag="stt_junk")
                    nc.vector.scalar_tensor_tensor(
                        sj[:], h_all[:, b, :], 1.0, vpb_sb[:, u, :],
                        op0=ALU.mult, op1=ALU.mult,
                        accum_out=pz[:, b, u:u + 1])

            # p = 128*sigmoid(z) = 64 + 64*tanh(z/2); round-half-even via MAGIC
            nc.scalar.activation(th[:, :, :], pz[:, :, :], AF.Tanh, scale=0.5)
            nc.vector.tensor_scalar(p_all[:, :, :], th[:, :, :], 64.0, 64.0,
                                    op0=ALU.mult, op1=ALU.add)
            nc.vector.tensor_scalar(rm_all[:, :, :], th[:, :, :], 64.0,
                                    64.0 + MAGIC, op0=ALU.mult, op1=ALU.add)
            nc.vector.tensor_scalar(rnd_all[:, :, :], rm_all[:, :, :], MAGIC,
                                    None, op0=ALU.subtract)

            # ---- index staging: rnd -> wrapped [16, m, (b,u)] in one trip
            nc.sync.dma_start(rnd_d[:, :], rnd_all[:, :, :])
            rnd_w = wk.tile([16, 8, 2 * NB], f32, tag="rnd_w")
            nc.sync.dma_start(
                rnd_w[:, :, :],
                bass.AP(rnd_d, 0, [[2 * NB, 16], [2 * NB * 16, 8], [1, 2 * NB]]))
            # base[16, m, b] = rnd_r*135 + rnd_c
            rw = rnd_w[:, :, :]
            base = wk.tile([16, 8, NB], f32, tag="base")
            nc.vector.scalar_tensor_tensor(
                base[:, :, :],
                ap_with(rw, [(2 * NB, 8), (2, NB)], 0), float(WP),
                ap_with(rw, [(2 * NB, 8), (2, NB)], 1),
                op0=ALU.mult, op1=ALU.add)
            # posf[16, b, i, m] = base[16, m, b] + 135*i
            ba = base[:, :, :]
            posf = wk.tile([16, NB, K7, 8], f32, tag="posf")
            nc.vector.tensor_tensor(
                posf[:, :, :, :],
                ap_with(io_sb[:], [(0, NB), (1, K7), (0, 8)]),
                ap_with(ba, [(1, NB), (0, K7), (NB, 8)]),
                op=ALU.add)
            posi = wk.tile([16, NB * K7 * 8], i32, tag="posi")
            nc.vector.tensor_copy(posi[:], posf[:, :, :, :])
            poss = wk.tile([16, NB * K7 * 8], i16, tag="poss")
            nc.vector.tensor_copy(poss[:], posi[:])
            # replicate to 8x16 partitions via DRAM (write 8 copies, read back)
            NF = NB * K7 * 8  # 448
            nc.sync.dma_start(
                bass.AP(idx_d, 0, [[NF, 16], [16 * NF, 8], [1, NF]]),
                ap_with(poss[:], [(0, 8), (1, NF)]))
            nc.sync.dma_start(idxw[:, :], idx_d[:, :])

            # ---- gathers: one per block, 896 rows of 7 contiguous pixels ----
            # row ids reach at most NPOS-7; count NPOS-6 keeps the declared
            # extent ((NPOS-7)*256 + 1792) exactly within the tensor.
            gsrc = bass.AP(qcl, 0, [[Cq, NPOS - 6], [1, ROWLEN]])
            patches = []
            for b in range(NB):
                patch = pp.tile([128, K7, ROWLEN], f16, tag="patch")
                nc.gpsimd.dma_gather(
                    patch[:, :, :], gsrc, idxw[:, b * 56:(b + 1) * 56],
                    TB * K7, TB * K7, ROWLEN, elem_step=Cq,
                    queue_num=b % 4)
                patches.append(patch)

            # ---- Wc = c_t @ W_a  (fp16 for the score path) ----
            for b in range(NB):
                blk = slice(b * 128, (b + 1) * 128)
                acc = ps.tile([128, Cq], f32, tag="mmwc")
                nc.tensor.matmul(acc[:], ct_sb[:, 0, blk], wa_sb[:, 0, :],
                                 start=True, stop=False)
                nc.tensor.matmul(acc[:], ct_sb[:, 1, blk], wa_sb[:, 1, :],
                                 start=False, stop=True)
                nc.scalar.copy(wc_sb[:, b, :], acc[:])

            # ---- validity + gaussian decay (batched over blocks) ----
            pa = p_all[:, :, :]
            ra = rnd_all[:, :, :]
            rc = []
            for u in range(2):
                p_u = ap_with(pa, [(2, NB), (0, K7)], u)
                rnd_u = ap_with(ra, [(2, NB), (0, K7)], u)
                cand = wk.tile([128, NB, K7], f32, tag=f"cand{u}")
                nc.vector.tensor_tensor(
                    cand[:, :, :], rnd_u,
                    ap_with(offs_sb[:], [(0, NB), (1, K7)]), op=ALU.add)
                ge = wk.tile([128, NB, K7], f32, tag=f"ge{u}")
                nc.vector.tensor_scalar(ge[:, :, :], cand[:, :, :], 1.0, None,
                                        op0=ALU.is_ge)
                le = wk.tile([128, NB, K7], f32, tag=f"le{u}")
                nc.vector.tensor_scalar(le[:, :, :], cand[:, :, :], float(H),
                                        None, op0=ALU.is_le)
                vv = wk.tile([128, NB, K7], f32, tag=f"vv{u}")
                nc.vector.tensor_tensor(vv[:, :, :], ge[:, :, :], le[:, :, :],
                                        op=ALU.mult)
                # d = (cand - 1) - p;  gexp = -(2/R^2) d^2
                d = wk.tile([128, NB, K7], f32, tag=f"d{u}")
                nc.vector.scalar_tensor_tensor(
                    d[:, :, :], cand[:, :, :], 1.0, p_u,
                    op0=ALU.subtract, op1=ALU.subtract)
                sq = wk.tile([128, NB, K7], f32, tag=f"sq{u}")
                nc.vector.tensor_tensor(sq[:, :, :], d[:, :, :], d[:, :, :],
                                        op=ALU.mult)
                gexp = wk.tile([128, NB, K7], f32, tag=f"gexp{u}")
                nc.vector.tensor_scalar(gexp[:, :, :], sq[:, :, :],
                                        -2.0 / (R * R), None, op0=ALU.mult)
                rc.append(dict(vv=vv, gexp=gexp))

            vv_r = rc[0]["vv"][:, :, :]
            vv_c = rc[1]["vv"][:, :, :]
            gx_r = rc[0]["gexp"][:, :, :]
            gx_c = rc[1]["gexp"][:, :, :]
            nc.vector.tensor_tensor(
                valid_sb[:, :, :].rearrange("p b (i j) -> p b i j", i=K7, j=K7),
                ap_with(vv_r, [(K7, NB), (1, K7), (0, K7)]),
                ap_with(vv_c, [(K7, NB), (0, K7), (1, K7)]),
                op=ALU.mult)
            nc.vector.tensor_tensor(
                elog_sb[:, :, :].rearrange("p b (i j) -> p b i j", i=K7, j=K7),
                ap_with(gx_r, [(K7, NB), (1, K7), (0, K7)]),
                ap_with(gx_c, [(K7, NB), (0, K7), (1, K7)]),
                op=ALU.add)

            # ---- per-block attention (3-stage software pipeline) ----
            wgts = {}
            scMs = {}
            ssums = {}

            def stageA(b):
                """scores + softmax -> wgt."""
                patch = patches[b]
                patchap = patch[:, :, :]

                # scores[t,k] = patch[t,k,:] . wc[t,:]
                # fp16 broadcast multiply, then in-place halving tree over c
                prod = prp.tile([128, K, Cq], f16, tag="prod")
                proda = prod[:, :, :]
                nc.vector.tensor_tensor(
                    ap_with(proda, [(Cq, K), (1, Cq)]),
                    ap_with(patchap, [(Cq, K), (1, Cq)]),
                    ap_with(wc_sb[:, b, :], [(0, K), (1, Cq)]),
                    op=ALU.mult)
                s = Cq // 2
                while s >= 2:
                    nc.vector.tensor_tensor(
                        ap_with(proda, [(Cq, K), (1, s)]),
                        ap_with(proda, [(Cq, K), (1, s)]),
                        ap_with(proda, [(Cq, K), (1, s)], s),
                        op=ALU.add)
                    s //= 2
                scores = wk.tile([128, K], f32, tag="scores")
                nc.vector.tensor_tensor(
                    ap_with(scores[:], [(1, K), (1, 1)]),
                    ap_with(proda, [(Cq, K), (1, 1)]),
                    ap_with(proda, [(Cq, K), (1, 1)], 1), op=ALU.add)

                # masked softmax: scM = (s + BIG) * valid
                scM = wk.tile([128, K], f32, tag="scM")
                nc.vector.scalar_tensor_tensor(
                    scM[:], scores[:], BIG, valid_sb[:, b, :],
                    op0=ALU.add, op1=ALU.mult)
                negm = wk.tile([128, 1], f32, tag="negm")
                nc.vector.tensor_reduce(negm[:], scM[:], axis=AX.X, op=ALU.max,
                                        negate=True)
                scMe = wk.tile([128, K], f32, tag="scMe")
                nc.vector.tensor_tensor(scMe[:], scM[:], elog_sb[:, b, :],
                                        op=ALU.add)
                scMs[b] = (scM, scMe, negm)

            def stageA2(b):
                """both exps on ACT: Z accumulation + unnormalized weights.

                wgt = exp(scM + elog - max) = softmax-numerator * expw; the
                1/Z normalization is applied to the final [p, Cq] sum in
                stageC, keeping DVE off the exp -> scale critical chain."""
                scM, scMe, negm = scMs.pop(b)
                e32 = wk.tile([128, K], f32, tag="e32")
                ssum = wk.tile([128, 1], f32, tag="ssum")
                nc.scalar.activation(e32[:], scM[:], AF.Exp, bias=negm[:],
                                     accum_out=ssum[:])
                wgt = wk.tile([128, K], f32, tag="wgt")
                nc.scalar.activation(wgt[:], scMe[:], AF.Exp, bias=negm[:])
                wgts[b] = wgt
                ssums[b] = ssum

            def pslice_of(patch, k):
                i, j = divmod(k, K7)
                return patch[:, i, j * Cq:(j + 1) * Cq]

            def stageB1(b):
                """first chunk of ACT combine scales (deps long ready)."""
                patch = patches[b]
                wgt = wgts[b]
                for k in range(CH1):
                    nc.scalar.activation(pslice_of(patch, k), pslice_of(patch, k),
                                         AF.Copy, scale=wgt[:, k:k + 1])

            def stageB2(b):
                """rest of combine scales: ACT tail + DVE tensor_scalar."""
                patch = patches[b]
                wgt = wgts.pop(b)
                for k in range(CH1, ACT_SLOTS):
                    nc.scalar.activation(pslice_of(patch, k), pslice_of(patch, k),
                                         AF.Copy, scale=wgt[:, k:k + 1])
                for k in range(ACT_SLOTS, K):
                    nc.vector.tensor_scalar(pslice_of(patch, k),
                                            pslice_of(patch, k),
                                            wgt[:, k:k + 1], None, op0=ALU.mult)

            def stageC(b):
                """tree reduce scaled patch over k: 49 -> 7 -> 1, write out."""
                patch = patches[b]
                patchap = patch[:, :, :]

                def wps(j0):
                    return ap_with(patchap, [(K7 * Cq, K7), (1, Cq)], j0 * Cq)

                ta = wk.tile([128, K7, Cq], f16, tag="ta")
                tb_ = wk.tile([128, K7, Cq], f16, tag="tb")
                tcc = wk.tile([128, K7, Cq], f16, tag="tc")
                nc.vector.tensor_tensor(ta[:, :, :], wps(0), wps(1), op=ALU.add)
                nc.vector.tensor_tensor(tb_[:, :, :], wps(2), wps(3), op=ALU.add)
                nc.vector.tensor_tensor(tcc[:, :, :], wps(4), wps(5), op=ALU.add)
                nc.vector.tensor_tensor(ta[:, :, :], ta[:, :, :], tb_[:, :, :],
                                        op=ALU.add)
                nc.vector.tensor_tensor(tcc[:, :, :], tcc[:, :, :], wps(6),
                                        op=ALU.add)
                nc.vector.tensor_tensor(ta[:, :, :], ta[:, :, :], tcc[:, :, :],
                                        op=ALU.add)
                # 7 -> 1
                taa = ta[:, :, :]
                t3 = wk.tile([128, 3, Cq], f16, tag="t3")
                nc.vector.tensor_tensor(
                    t3[:, :, :],
                    ap_with(taa, [(2 * Cq, 3), (1, Cq)], 0),
                    ap_with(taa, [(2 * Cq, 3), (1, Cq)], Cq),
                    op=ALU.add)
                t2 = wk.tile([128, Cq], f16, tag="t2")
                nc.vector.tensor_tensor(t2[:], t3[:, 0, :], t3[:, 1, :],
                                        op=ALU.add)
                t1 = wk.tile([128, Cq], f16, tag="t1")
                nc.vector.tensor_tensor(t1[:], t3[:, 2, :], ta[:, 6, :],
                                        op=ALU.add)
                rinv = wk.tile([128, 1], f32, tag="rinv")
                nc.vector.reciprocal(rinv[:], ssums.pop(b)[:])
                osum = wk.tile([128, Cq], f32, tag="osum")
                nc.vector.tensor_tensor(osum[:], t2[:], t1[:], op=ALU.add)
                oacc = wk.tile([128, Cq], f32, tag="oacc")
                nc.vector.tensor_scalar(oacc[:], osum[:], rinv[:], None,
                                        op0=ALU.mult)
                nc.sync.dma_start(out_d[b * 128:(b + 1) * 128, :], oacc[:])

            for it in range(NB + 2):
                if it < NB:
                    stageA(it)
                if 1 <= it <= NB:
                    stageB1(it - 1)
                if it < NB:
                    stageA2(it)
                if 1 <= it <= NB:
                    stageB2(it - 1)
                if it >= 2:
                    stageC(it - 2)

    nc.compile()
    return nc


def _prep_inputs(q, c_t, W_a, W_p, V_p):
    """Build per-core input maps (host-side sharding + layout prep)."""
    q = np.ascontiguousarray(q, dtype=np.float32)
    c_t = np.ascontiguousarray(c_t, dtype=np.float32)
    W_a = np.ascontiguousarray(W_a, dtype=np.float32)
    W_p = np.ascontiguousarray(W_p, dtype=np.float32)
    V_p = np.ascontiguousarray(V_p, dtype=np.float32)

    wa = W_a                                   # [Cc, Cq]
    wpT = W_p.T.copy()                         # [Cc, P]
    vpb = np.repeat(V_p[:, None, :], P, axis=1).reshape(2 * P, P).copy()
    offs = np.tile(np.arange(-2, 5, dtype=np.float32), (128, 1))
    io135 = np.tile(np.arange(K7, dtype=np.float32) * WP, (16, 1))

    in_maps = []
    for b in range(B):
        qcl = np.zeros((HP, WP, Cq), dtype=np.float16)
        qcl[PADL:PADL + H, PADL:PADL + W, :] = np.transpose(
            q[b], (1, 2, 0)).astype(np.float16)
        in_maps.append(dict(
            qcl=qcl.reshape(NPOS, Cq),
            ctT=c_t[b].T.copy(),
            wa=wa, wpT=wpT, vpb=vpb, offs=offs, io135=io135,
        ))
    return in_maps


def run(q, c_t, W_a, W_p, V_p, trace=False, **spmd_kwargs):
    if "nc" not in _CACHE:
        _CACHE["nc"] = _build_nc()
    nc = _CACHE["nc"]
    in_maps = _prep_inputs(q, c_t, W_a, W_p, V_p)
    res = run_bass_kernel_spmd(nc, in_maps, core_ids=list(range(B)),
                               trace=trace, **spmd_kwargs)
    out = np.stack([res.results[b]["out"] for b in range(B)], axis=0)
    return out, res


def kernel(q, c_t, W_a, W_p, V_p):
    out, _ = run(q, c_t, W_a, W_p, V_p, trace=False)
    return out


# revision 44
# speedup vs baseline: 1.0440x; 1.0440x over previous
"""
Trainium2 Bass kernel for nn_LocalAttention2d (sparse local attention with
predictive alignment).

Strategy (pure data parallel, B=8 batches over 8 NeuronCores):
  host:   per batch, build a zero-padded channels-last fp16 image
          qcl[(H+7)*(W+7), Cq]; transpose c_t -> [Cc, T].
  device: PE computes Wc = c_t @ W_a and the alignment MLP p_t
          (sigmoid via tanh identity to stay in one ACT table set).
          Window-row indices are computed directly in the SWDGE wrapped
          [16, ...] index layout via one small DRAM round trip.  One
          dma_gather descriptor fetches a whole 7-pixel window row
          (elem_step=256 < elem_size=1792, rows overlap in DRAM), so each
          128-query block needs just 896 descriptors of 3.5KB.
          Scores: fused scalar_tensor_tensor multiply + accumulate per
          window slot (fp16 data, fp32 accumulation).  Masked softmax via
          the (s+BIG)*valid trick.  Combine: per-slot scaled copies split
          between ACT (activation scale) and DVE (tensor_scalar), then a
          strided pairwise tree reduction on DVE, all fp16.
"""

import numpy as np

import concourse.bass as bass
import concourse.bacc as bacc
import concourse.mybir as mybir
import concourse.tile as tile
from concourse.bass_utils import run_bass_kernel_spmd

f32 = mybir.dt.float32
f16 = mybir.dt.float16
i32 = mybir.dt.int32
i16 = mybir.dt.int16
AF = mybir.ActivationFunctionType
ALU = mybir.AluOpType
AX = mybir.AxisListType

# Problem constants (hardcoded per contract)
B, Cq, H, W = 8, 256, 128, 128
T, Cc, P = 1024, 256, 128
R = 3
K7 = 2 * R + 1          # 7
K = K7 * K7             # 49
PADL = 3                # original pixel (y,x) -> padded (y+3, x+3)
HP = H + K7             # 135
WP = W + K7             # 135
NPOS = HP * WP          # 18225
TB = 128                # queries per block
NB = T // TB            # 8 blocks
ROWLEN = K7 * Cq        # 1792 elems per gathered window row
MAGIC = 8388608.0       # 2^23: fp32 add/sub rounds half-even like jnp.round
BIG = 1024.0            # mask offset; exp(-~1000) == 0
ACT_SLOTS = 44          # combine slots scaled on ACT; rest on DVE
CH1 = 25                # ACT slots emitted before the next block's exp

_CACHE = {}


def _build_nc():
    nc = bacc.Bacc(None, target_bir_lowering=False, num_swdge_queues=4)

    qcl = nc.dram_tensor("qcl", [NPOS, Cq], f16, kind="ExternalInput")
    ctT = nc.dram_tensor("ctT", [Cc, T], f32, kind="ExternalInput")
    wa = nc.dram_tensor("wa", [Cc, Cq], f32, kind="ExternalInput")
    wpT = nc.dram_tensor("wpT", [Cc, P], f32, kind="ExternalInput")
    vpb = nc.dram_tensor("vpb", [2 * P, P], f32, kind="ExternalInput")
    offs = nc.dram_tensor("offs", [128, K7], f32, kind="ExternalInput")
    io135 = nc.dram_tensor("io135", [16, K7], f32, kind="ExternalInput")
    out_d = nc.dram_tensor("out", [T, Cq], f32, kind="ExternalOutput")
    rnd_d = nc.dram_tensor("rnd_stage", [128, 2 * NB], f32, kind="Internal")
    idx_d = nc.dram_tensor("idx_stage", [128, NB * K7 * 8], i16, kind="Internal")

    def ap_with(apx, dims, doff=0):
        """AP reusing apx's partition dim, explicit free dims, +elem offset."""
        return bass.AP(apx.tensor, apx.offset + doff,
                       [apx.ap[0]] + [list(d) for d in dims])

    with tile.TileContext(nc) as tc:
        with (
            tc.tile_pool(name="const", bufs=1) as cp,
            tc.tile_pool(name="work", bufs=2) as wk,
            tc.tile_pool(name="patch", bufs=4) as pp,
            tc.tile_pool(name="prodp", bufs=1) as prp,
            tc.tile_pool(name="psum", bufs=4, space="PSUM") as ps,
        ):
            # ---- load weights/constants ----
            ct_sb = cp.tile([128, 2, T], f32)
            wa_sb = cp.tile([128, 2, Cq], f32)
            wpT_sb = cp.tile([128, 2, P], f32)
            vpb_sb = cp.tile([128, 2, P], f32)
            offs_sb = cp.tile([128, K7], f32)
            io_sb = cp.tile([16, K7], f32)
            for h in range(2):
                nc.sync.dma_start(ct_sb[:, h, :], ctT[h * 128:(h + 1) * 128, :])
                nc.sync.dma_start(wpT_sb[:, h, :], wpT[h * 128:(h + 1) * 128, :])
                nc.sync.dma_start(wa_sb[:, h, :], wa[h * 128:(h + 1) * 128, :])
                nc.sync.dma_start(vpb_sb[:, h, :], vpb[h * 128:(h + 1) * 128, :])
            nc.sync.dma_start(offs_sb[:], offs[:])
            nc.sync.dma_start(io_sb[:], io135[:])

            # persistent per-block state
            wc_sb = cp.tile([128, NB, Cq], f16)      # Wc per block (fp16)
            valid_sb = cp.tile([128, NB, K], f32)    # slot validity
            expw_sb = cp.tile([128, NB, K], f32)     # gaussian decay
            h_all = cp.tile([128, NB, P], f32)       # tanh(c_t @ W_p.T)
            pz = cp.tile([128, NB, 2], f32)          # raw V_p dots
            th = cp.tile([128, NB, 2], f32)          # tanh(z/2)
            p_all = cp.tile([128, NB, 2], f32)       # p_t
            rm_all = cp.tile([128, NB, 2], f32)      # p + MAGIC
            rnd_all = cp.tile([128, NB, 2], f32)     # round(p)
            idxw = cp.tile([128, NB * K7 * 8], i16)  # wrapped gather indices

            # ---- alignment MLP: h = tanh(ct @ WpT), z = h . Vp ----
            for b in range(NB):
                blk = slice(b * 128, (b + 1) * 128)
                acch = ps.tile([128, P], f32, tag="mmh")
                nc.tensor.matmul(acch[:], ct_sb[:, 0, blk], wpT_sb[:, 0, :],
                                 start=True, stop=False)
                nc.tensor.matmul(acch[:], ct_sb[:, 1, blk], wpT_sb[:, 1, :],
                                 start=False, stop=True)
                nc.scalar.activation(h_all[:, b, :], acch[:], AF.Tanh)
                for u in range(2):
                    sj = wk.tile([128, P], f16, tag="stt_junk")
                    nc.vector.scalar_tensor_tensor(
                        sj[:], h_all[:, b, :], 1.0, vpb_sb[:, u, :],
                        op0=ALU.mult, op1=ALU.mult,
                        accum_out=pz[:, b, u:u + 1])

            # p = 128*sigmoid(z) = 64 + 64*tanh(z/2); round-half-even via MAGIC
            nc.scalar.activation(th[:, :, :], pz[:, :, :], AF.Tanh, scale=0.5)
            nc.vector.tensor_scalar(p_all[:, :, :], th[:, :, :], 64.0, 64.0,
                                    op0=ALU.mult, op1=ALU.add)
            nc.vector.tensor_scalar(rm_all[:, :, :], th[:, :, :], 64.0,
                                    64.0 + MAGIC, op0=ALU.mult, op1=ALU.add)
            nc.vector.tensor_scalar(rnd_all[:, :, :], rm_all[:, :, :], MAGIC,
                                    None, op0=ALU.subtract)

            # ---- index staging: rnd -> wrapped [16, m, (b,u)] in one trip
            nc.sync.dma_start(rnd_d[:, :], rnd_all[:, :, :])
            rnd_w = wk.tile([16, 8, 2 * NB], f32, tag="rnd_w")
            nc.sync.dma_start(
                rnd_w[:, :, :],
                bass.AP(rnd_d, 0, [[2 * NB, 16], [2 * NB * 16, 8], [1, 2 * NB]]))
            # base[16, m, b] = rnd_r*135 + rnd_c
            rw = rnd_w[:, :, :]
            base = wk.tile([16, 8, NB], f32, tag="base")
            nc.vector.scalar_tensor_tensor(
                base[:, :, :],
                ap_with(rw, [(2 * NB, 8), (2, NB)], 0), float(WP),
                ap_with(rw, [(2 * NB, 8), (2, NB)], 1),
                op0=ALU.mult, op1=ALU.add)
            # posf[16, b, i, m] = base[16, m, b] + 135*i
            ba = base[:, :, :]
            posf = wk.tile([16, NB, K7, 8], f32, tag="posf")
            nc.vector.tensor_tensor(
                posf[:, :, :, :],
                ap_with(io_sb[:], [(0, NB), (1, K7), (0, 8)]),
                ap_with(ba, [(1, NB), (0, K7), (NB, 8)]),
                op=ALU.add)
            posi = wk.tile([16, NB * K7 * 8], i32, tag="posi")
            nc.vector.tensor_copy(posi[:], posf[:, :, :, :])
            poss = wk.tile([16, NB * K7 * 8], i16, tag="poss")
            nc.vector.tensor_copy(poss[:], posi[:])
            # replicate to 8x16 partitions via DRAM (write 8 copies, read back)
            NF = NB * K7 * 8  # 448
            nc.sync.dma_start(
                bass.AP(idx_d, 0, [[NF, 16], [16 * NF, 8], [1, NF]]),
                ap_with(poss[:], [(0, 8), (1, NF)]))
            nc.sync.dma_start(idxw[:, :], idx_d[:, :])

            # ---- gathers: one per block, 896 rows of 7 contiguous pixels ----
            # row ids reach at most NPOS-7; count NPOS-6 keeps the declared
            # extent ((NPOS-7)*256 + 1792) exactly within the tensor.
            gsrc = bass.AP(qcl, 0, [[Cq, NPOS - 6], [1, ROWLEN]])
            patches = []
            for b in range(NB):
                patch = pp.tile([128, K7, ROWLEN], f16, tag="patch")
                nc.gpsimd.dma_gather(
                    patch[:, :, :], gsrc, idxw[:, b * 56:(b + 1) * 56],
                    TB * K7, TB * K7, ROWLEN, elem_step=Cq,
                    queue_num=b % 4)
                patches.append(patch)

            # ---- Wc = c_t @ W_a  (fp16 for the score path) ----
            for b in range(NB):
                blk = slice(b * 128, (b + 1) * 128)
                acc = ps.tile([128, Cq], f32, tag="mmwc")
                nc.tensor.matmul(acc[:], ct_sb[:, 0, blk], wa_sb[:, 0, :],
                                 start=True, stop=False)
                nc.tensor.matmul(acc[:], ct_sb[:, 1, blk], wa_sb[:, 1, :],
                                 start=False, stop=True)
                nc.scalar.copy(wc_sb[:, b, :], acc[:])

            # ---- validity + gaussian decay (batched over blocks) ----
            pa = p_all[:, :, :]
            ra = rnd_all[:, :, :]
            rc = []
            for u in range(2):
                p_u = ap_with(pa, [(2, NB), (0, K7)], u)
                rnd_u = ap_with(ra, [(2, NB), (0, K7)], u)
                cand = wk.tile([128, NB, K7], f32, tag=f"cand{u}")
                nc.vector.tensor_tensor(
                    cand[:, :, :], rnd_u,
                    ap_with(offs_sb[:], [(0, NB), (1, K7)]), op=ALU.add)
                ge = wk.tile([128, NB, K7], f32, tag=f"ge{u}")
                nc.vector.tensor_scalar(ge[:, :, :], cand[:, :, :], 1.0, None,
                                        op0=ALU.is_ge)
                le = wk.tile([128, NB, K7], f32, tag=f"le{u}")
                nc.vector.tensor_scalar(le[:, :, :], cand[:, :, :], float(H),
                                        None, op0=ALU.is_le)
                vv = wk.tile([128, NB, K7], f32, tag=f"vv{u}")
                nc.vector.tensor_tensor(vv[:, :, :], ge[:, :, :], le[:, :, :],
                                        op=ALU.mult)
                # d = (cand - 1) - p;  gexp = -(2/R^2) d^2
                d = wk.tile([128, NB, K7], f32, tag=f"d{u}")
                nc.vector.scalar_tensor_tensor(
                    d[:, :, :], cand[:, :, :], 1.0, p_u,
                    op0=ALU.subtract, op1=ALU.subtract)
                sq = wk.tile([128, NB, K7], f32, tag=f"sq{u}")
                nc.vector.tensor_tensor(sq[:, :, :], d[:, :, :], d[:, :, :],
                                        op=ALU.mult)
                gexp = wk.tile([128, NB, K7], f32, tag=f"gexp{u}")
                nc.vector.tensor_scalar(gexp[:, :, :], sq[:, :, :],
                                        -2.0 / (R * R), None, op0=ALU.mult)
                rc.append(dict(vv=vv, gexp=gexp))

            vv_r = rc[0]["vv"][:, :, :]
            vv_c = rc[1]["vv"][:, :, :]
            gx_r = rc[0]["gexp"][:, :, :]
            gx_c = rc[1]["gexp"][:, :, :]
            nc.vector.tensor_tensor(
                valid_sb[:, :, :].rearrange("p b (i j) -> p b i j", i=K7, j=K7),
                ap_with(vv_r, [(K7, NB), (1, K7), (0, K7)]),
                ap_with(vv_c, [(K7, NB), (0, K7), (1, K7)]),
                op=ALU.mult)
            elog = wk.tile([128, NB, K], f32, tag="elog")
            nc.vector.tensor_tensor(
                elog[:, :, :].rearrange("p b (i j) -> p b i j", i=K7, j=K7),
                ap_with(gx_r, [(K7, NB), (1, K7), (0, K7)]),
                ap_with(gx_c, [(K7, NB), (0, K7), (1, K7)]),
                op=ALU.add)
            nc.scalar.activation(expw_sb[:, :, :], elog[:, :, :], AF.Exp)

            # ---- per-block attention (3-stage software pipeline) ----
            wgts = {}
            scMs = {}
            ssums = {}

            def stageA(b):
                """scores + softmax -> wgt."""
                patch = patches[b]
                patchap = patch[:, :, :]

                # scores[t,k] = patch[t,k,:] . wc[t,:]
                # fp16 broadcast multiply, then in-place halving tree over c
                prod = prp.tile([128, K, Cq], f16, tag="prod")
                proda = prod[:, :, :]
                nc.vector.tensor_tensor(
                    ap_with(proda, [(Cq, K), (1, Cq)]),
                    ap_with(patchap, [(Cq, K), (1, Cq)]),
                    ap_with(wc_sb[:, b, :], [(0, K), (1, Cq)]),
                    op=ALU.mult)
                s = Cq // 2
                while s >= 2:
                    nc.vector.tensor_tensor(
                        ap_with(proda, [(Cq, K), (1, s)]),
                        ap_with(proda, [(Cq, K), (1, s)]),
                        ap_with(proda, [(Cq, K), (1, s)], s),
                        op=ALU.add)
                    s //= 2
                scores = wk.tile([128, K], f32, tag="scores")
                nc.vector.tensor_tensor(
                    ap_with(scores[:], [(1, K), (1, 1)]),
                    ap_with(proda, [(Cq, K), (1, 1)]),
                    ap_with(proda, [(Cq, K), (1, 1)], 1), op=ALU.add)

                # masked softmax: scM = (s + BIG) * valid
                scM = wk.tile([128, K], f32, tag="scM")
                nc.vector.scalar_tensor_tensor(
                    scM[:], scores[:], BIG, valid_sb[:, b, :],
                    op0=ALU.add, op1=ALU.mult)
                negm = wk.tile([128, 1], f32, tag="negm")
                nc.vector.tensor_reduce(negm[:], scM[:], axis=AX.X, op=ALU.max,
                                        negate=True)
                scMs[b] = (scM, negm)

            def stageA2(b):
                """exp (ACT) + normalization -> wgt (DVE)."""
                scM, negm = scMs.pop(b)
                e32 = wk.tile([128, K], f32, tag="e32")
                ssum = wk.tile([128, 1], f32, tag="ssum")
                nc.scalar.activation(e32[:], scM[:], AF.Exp, bias=negm[:],
                                     accum_out=ssum[:])
                rinv = wk.tile([128, 1], f32, tag="rinv")
                nc.vector.reciprocal(rinv[:], ssum[:])
                wgt = wk.tile([128, K], f32, tag="wgt")
                nc.vector.scalar_tensor_tensor(
                    wgt[:], e32[:], rinv[:], expw_sb[:, b, :],
                    op0=ALU.mult, op1=ALU.mult)
                wgts[b] = wgt

            def pslice_of(patch, k):
                i, j = divmod(k, K7)
                return patch[:, i, j * Cq:(j + 1) * Cq]

            def stageB1(b):
                """first chunk of ACT combine scales (deps long ready)."""
                patch = patches[b]
                wgt = wgts[b]
                for k in range(CH1):
                    nc.scalar.activation(pslice_of(patch, k), pslice_of(patch, k),
                                         AF.Copy, scale=wgt[:, k:k + 1])

            def stageB2(b):
                """rest of combine scales: ACT tail + DVE tensor_scalar."""
                patch = patches[b]
                wgt = wgts.pop(b)
                for k in range(CH1, ACT_SLOTS):
                    nc.scalar.activation(pslice_of(patch, k), pslice_of(patch, k),
                                         AF.Copy, scale=wgt[:, k:k + 1])
                for k in range(ACT_SLOTS, K):
                    nc.vector.tensor_scalar(pslice_of(patch, k),
                                            pslice_of(patch, k),
                                            wgt[:, k:k + 1], None, op0=ALU.mult)

            def stageC(b):
                """tree reduce scaled patch over k: 49 -> 7 -> 1, write out."""
                patch = patches[b]
                patchap = patch[:, :, :]

                def wps(j0):
                    return ap_with(patchap, [(K7 * Cq, K7), (1, Cq)], j0 * Cq)

                ta = wk.tile([128, K7, Cq], f16, tag="ta")
                tb_ = wk.tile([128, K7, Cq], f16, tag="tb")
                tcc = wk.tile([128, K7, Cq], f16, tag="tc")
                nc.vector.tensor_tensor(ta[:, :, :], wps(0), wps(1), op=ALU.add)
                nc.vector.tensor_tensor(tb_[:, :, :], wps(2), wps(3), op=ALU.add)
                nc.vector.tensor_tensor(tcc[:, :, :], wps(4), wps(5), op=ALU.add)
                nc.vector.tensor_tensor(ta[:, :, :], ta[:, :, :], tb_[:, :, :],
                                        op=ALU.add)
                nc.vector.tensor_tensor(tcc[:, :, :], tcc[:, :, :], wps(6),
                                        op=ALU.add)
                nc.vector.tensor_tensor(ta[:, :, :], ta[:, :, :], tcc[:, :, :],
                                        op=ALU.add)
                # 7 -> 1
                taa = ta[:, :, :]
                t3 = wk.tile([128, 3, Cq], f16, tag="t3")
                nc.vector.tensor_tensor(
                    t3[:, :, :],
                    ap_with(taa, [(2 * Cq, 3), (1, Cq)], 0),
                    ap_with(taa, [(2 * Cq, 3), (1, Cq)], Cq),
                    op=ALU.add)
                t2 = wk.tile([128, Cq], f16, tag="t2")
                nc.vector.tensor_tensor(t2[:], t3[:, 0, :], t3[:, 1, :],
                                        op=ALU.add)
                t1 = wk.tile([128, Cq], f16, tag="t1")
                nc.vector.tensor_tensor(t1[:], t3[:, 2, :], ta[:, 6, :],
                                        op=ALU.add)
                oacc = wk.tile([128, Cq], f32, tag="oacc")
                nc.vector.tensor_tensor(oacc[:], t2[:], t1[:], op=ALU.add)
                nc.sync.dma_start(out_d[b * 128:(b + 1) * 128, :], oacc[:])

            for it in range(NB + 2):
                if it < NB:
                    stageA(it)
                if 1 <= it <= NB:
                    stageB1(it - 1)
                if it < NB:
                    stageA2(it)
                if 1 <= it <= NB:
                    stageB2(it - 1)
                if it >= 2:
                    stageC(it - 2)

    nc.compile()
    return nc


def _prep_inputs(q, c_t, W_a, W_p, V_p):
    """Build per-core input maps (host-side sharding + layout prep)."""
    q = np.ascontiguousarray(q, dtype=np.float32)
    c_t = np.ascontiguousarray(c_t, dtype=np.float32)
    W_a = np.ascontiguousarray(W_a, dtype=np.float32)
    W_p = np.ascontiguousarray(W_p, dtype=np.float32)
    V_p = np.ascontiguousarray(V_p, dtype=np.float32)

    wa = W_a                                   # [Cc, Cq]
    wpT = W_p.T.copy()                         # [Cc, P]
    vpb = np.repeat(V_p[:, None, :], P, axis=1).reshape(2 * P, P).copy()
    offs = np.tile(np.arange(-2, 5, dtype=np.float32), (128, 1))
    io135 = np.tile(np.arange(K7, dtype=np.float32) * WP, (16, 1))

    in_maps = []
    for b in range(B):
        qcl = np.zeros((HP, WP, Cq), dtype=np.float16)
        qcl[PADL:PADL + H, PADL:PADL + W, :] = np.transpose(
            q[b], (1, 2, 0)).astype(np.float16)
        in_maps.append(dict(
            qcl=qcl.reshape(NPOS, Cq),
            ctT=c_t[b].T.copy(),
            wa=wa, wpT=wpT, vpb=vpb, offs=offs, io135=io135,
        ))
    return in_maps


def run(q, c_t, W_a, W_p, V_p, trace=False, **spmd_kwargs):
    if "nc" not in _CACHE:
        _CACHE["nc"] = _build_nc()
    nc = _CACHE["nc"]
    in_maps = _prep_inputs(q, c_t, W_a, W_p, V_p)
    res = run_bass_kernel_spmd(nc, in_maps, core_ids=list(range(B)),
                               trace=trace, **spmd_kwargs)
    out = np.stack([res.results[b]["out"] for b in range(B)], axis=0)
    return out, res


def kernel(q, c_t, W_a, W_p, V_p):
    out, _ = run(q, c_t, W_a, W_p, V_p, trace=False)
    return out
